# revision 1
# baseline (speedup 1.0000x reference)
"""CHSLoss (topk_masking) Trainium2 Bass kernel.

Data-parallel over batch: 8 cores x 4 images each. Per core:
  - 8x8 block-sum pooling of gt_density via PE matmuls (per-chunk block
    indicator lhsT accumulated into PSUM), then a strided DVE reduce.
  - dg shuffled into a [16 partitions x 1024] per-image "row" layout so that
    each loss row (image) owns a 16-partition group.
  - top-k threshold per row found by fixed-round bisection on squared errors:
    count(E >= mid) computed with tensor_scalar+accum, partition-group sums
    via the DVE 32x32 stream-transpose trick.
  - masked MSE reduced to per-partition partials; host sums the 8x128 partials.
"""

import numpy as np

import concourse.bacc as bacc
import concourse.tile as tile
from concourse import mybir
from concourse.bass_utils import run_bass_kernel_spmd

F32 = mybir.dt.float32
ALU = mybir.AluOpType

N_CORES = 8
B, C, H, W = 32, 1, 128, 128
SIZE = 8
GH, GW = H * SIZE, W * SIZE  # 1024, 1024
IMGS_PER_CORE = B // N_CORES  # 4
N_ROW = H * W  # elements per loss row (16384)
MAX_NOISY_RATIO = 0.1
MAX_WEIGHT_RATIO = 1.0
R_BISECT = 22  # bisection rounds; rel err ~4e-6 at 22 (verified vs reference)

_cache: dict = {}


def _build_program(num: int, weight: float):
    """Build + compile the per-core Bass program. Same program on all cores."""
    nc = bacc.Bacc("TRN2", target_bir_lowering=False, debug=False,
                   num_devices=N_CORES)

    gt = nc.declare_dram_parameter("gt", [IMGS_PER_CORE, GH, GW], F32,
                                   isOutput=False)
    dcp = nc.declare_dram_parameter("dc", [IMGS_PER_CORE, H, W], F32,
                                    isOutput=False)
    dtp = nc.declare_dram_parameter("dt", [IMGS_PER_CORE, H, W], F32,
                                    isOutput=False)
    ind = nc.declare_dram_parameter("ind", [8 * 128, 128], F32, isOutput=False)
    w16 = nc.declare_dram_parameter("w16", [128, 32], F32, isOutput=False)
    accp_out = nc.declare_dram_parameter("accp", [128, 1], F32, isOutput=True)

    with tile.TileContext(nc) as tc:
        with (
            tc.tile_pool(name="imgs", bufs=2) as imgpool,
            tc.tile_pool(name="psum", bufs=2, space="PSUM") as psumpool,
            tc.tile_pool(name="consts", bufs=1) as constpool,
            tc.tile_pool(name="work", bufs=1) as work,
            tc.tile_pool(name="dg", bufs=2) as dgpool,
            tc.tile_pool(name="small", bufs=1) as small,
        ):
            indt = constpool.tile([128, 8, 128], F32)
            nc.sync.dma_start(indt[:], ind.rearrange("(o k) m -> k o m", o=8))
            w16t = constpool.tile([128, 32], F32)
            nc.sync.dma_start(w16t[:], w16[:])

            G = work.tile([128, 8 * 128], F32)
            S = work.tile([128, 8 * 128], F32)
            Sp = work.tile([128, 8 * 128], F32)

            for i in range(IMGS_PER_CORE):
                # gt image: partition k holds rows {128*o + k}, free (o, w).
                # Two half-image DMAs so matmuls start at the half mark.
                img_t = imgpool.tile([128, 8, GW], F32)
                gt_i = gt[i].rearrange("(o k) w -> k o w", o=8, k=128)
                nc.sync.dma_start(img_t[:, 0:4, :], gt_i[:, 0:4, :])
                nc.sync.dma_start(img_t[:, 4:8, :], gt_i[:, 4:8, :])

                # Stage A (row-group sums) on PE: chunk o's indicator places
                # its 16 hh-groups at out partitions [16o, 16o+16).
                rs = psumpool.tile([128, GW], F32)
                for half in range(2):
                    sl = slice(half * 512, (half + 1) * 512)
                    for o in range(8):
                        nc.tensor.matmul(
                            rs[:, sl],
                            indt[:, o, :],
                            img_t[:, o, sl],
                            start=(o == 0),
                            stop=(o == 7),
                        )

                # Stage B: sum each 8-wide column group -> dg [128(hh), 128(ww)]
                dg = dgpool.tile([128, 128], F32)
                nc.vector.tensor_reduce(
                    dg[:],
                    rs[:].rearrange("p (w j) -> p w j", j=8),
                    axis=mybir.AxisListType.X,
                    op=ALU.add,
                )

                # Shuffle dg -> G row block: G[32i+q, r*128+w] = dg[8q+r, w]
                gslot = G[32 * i : 32 * i + 16, :]
                for r in range(8):
                    nc.sync.dma_start(
                        gslot[:, r * 128 : (r + 1) * 128],
                        dg[r : r + 121 : 8, :],
                    )
                nc.sync.dma_start(G[32 * i + 16 : 32 * i + 32, :], gslot[:])

                # dmap rows in the same [16, 1024] layout
                dc_i = dcp[i].rearrange("(q r) w -> q r w", q=16)
                dt_i = dtp[i].rearrange("(q r) w -> q r w", q=16)
                s_lo = S[32 * i : 32 * i + 16, :].rearrange(
                    "q (r w) -> q r w", r=8)
                s_hi = S[32 * i + 16 : 32 * i + 32, :].rearrange(
                    "q (r w) -> q r w", r=8)
                sp_lo = Sp[32 * i : 32 * i + 16, :].rearrange(
                    "q (r w) -> q r w", r=8)
                sp_hi = Sp[32 * i + 16 : 32 * i + 32, :].rearrange(
                    "q (r w) -> q r w", r=8)
                nc.sync.dma_start(s_lo, dc_i)
                nc.sync.dma_start(s_hi, dt_i)
                nc.sync.dma_start(sp_lo, dt_i)
                nc.sync.dma_start(sp_hi, dc_i)

            # A = S - G ; Bw = weight * (Sp - G) ; E = A*A
            A = work.tile([128, 1024], F32)
            Bw = work.tile([128, 1024], F32)
            E = work.tile([128, 1024], F32)
            nc.vector.tensor_tensor(out=A[:], in0=S[:], in1=G[:],
                                    op=ALU.subtract)
            nc.vector.tensor_tensor(out=Bw[:], in0=Sp[:], in1=G[:],
                                    op=ALU.subtract)
            if weight != 1.0:
                nc.vector.tensor_scalar(out=Bw[:], in0=Bw[:],
                                        scalar1=float(weight), scalar2=None,
                                        op0=ALU.mult)
            nc.vector.tensor_tensor(out=E[:], in0=A[:], in1=A[:], op=ALU.mult)

            cjunk = work.tile([128, 1024], F32)
            thr = small.tile([128, 1], F32)
            if num >= 1:
                mid = small.tile([128, 1], F32)
                cnt = small.tile([128, 1], F32)
                tj = small.tile([128, 32], F32)
                gj = small.tile([128, 32], F32)
                gcnt = small.tile([128, 1], F32)
                delta = small.tile([128, 1], F32)
                nc.vector.memset(mid[:], 2048.0)
                for r in range(R_BISECT):
                    # cnt[p] = #(E[p,:] >= mid[p])
                    nc.vector.tensor_scalar(
                        out=cjunk[:], in0=E[:], scalar1=mid[:], scalar2=0.0,
                        op0=ALU.is_ge, op1=ALU.add, accum_out=cnt[:],
                    )
                    # group-sum cnt over 16-partition blocks, broadcast back
                    nc.vector.transpose(tj[:], cnt[:].to_broadcast([128, 32]))
                    nc.vector.tensor_tensor(out=gj[:], in0=tj[:], in1=w16t[:],
                                            op=ALU.mult)
                    nc.vector.tensor_reduce(gcnt[:], gj[:],
                                            axis=mybir.AxisListType.X,
                                            op=ALU.add)
                    step = float(1024.0 * 2.0 ** (-r))
                    # delta = (gcnt >= num) * 2*step ; mid += delta - step
                    nc.vector.tensor_scalar(
                        out=delta[:], in0=gcnt[:], scalar1=float(num),
                        scalar2=2.0 * step, op0=ALU.is_ge, op1=ALU.mult,
                    )
                    nc.vector.scalar_tensor_tensor(
                        out=mid[:], in0=mid[:], scalar=-step, in1=delta[:],
                        op0=ALU.add, op1=ALU.add,
                    )
                nc.vector.tensor_scalar(
                    out=thr[:], in0=mid[:],
                    scalar1=float(4096.0 * 2.0 ** (-R_BISECT)), scalar2=None,
                    op0=ALU.subtract,
                )
            else:
                nc.vector.memset(thr[:], 3.0e38)

            # t1 = (E >= thr) * Bw ; d = A - t1 ; accp = sum(d*d) per partition
            t1 = work.tile([128, 1024], F32)
            nc.vector.scalar_tensor_tensor(
                out=t1[:], in0=E[:], scalar=thr[:], in1=Bw[:],
                op0=ALU.is_ge, op1=ALU.mult,
            )
            d = cjunk  # reuse
            nc.vector.tensor_tensor(out=d[:], in0=A[:], in1=t1[:],
                                    op=ALU.subtract)
            dsq = t1  # reuse
            accp = small.tile([128, 1], F32)
            nc.vector.scalar_tensor_tensor(
                out=dsq[:], in0=d[:], scalar=0.0, in1=d[:],
                op0=ALU.add, op1=ALU.mult, accum_out=accp[:],
            )
            nc.sync.dma_start(accp_out[:], accp[:])

    nc.compile()
    return nc


def _constants():
    ind_np = np.zeros((8, 128, 128), dtype=np.float32)
    for o in range(8):
        for k in range(128):
            ind_np[o, k, 16 * o + k // 8] = 1.0
    w16_np = np.zeros((128, 32), dtype=np.float32)
    for p in range(128):
        w16_np[p, 16 * ((p // 16) % 2) : 16 * ((p // 16) % 2) + 16] = 1.0
    return ind_np.reshape(8 * 128, 128), w16_np


def kernel(dmap_conv, dmap_tran, gt_density, process):
    dmap_conv = np.asarray(dmap_conv, dtype=np.float32).reshape(B, H, W)
    dmap_tran = np.asarray(dmap_tran, dtype=np.float32).reshape(B, H, W)
    gt_density = np.asarray(gt_density, dtype=np.float32).reshape(B, GH, GW)
    p = float(np.asarray(process))

    weight = MAX_WEIGHT_RATIO * p
    noisy_ratio = MAX_NOISY_RATIO * p
    num = int(H * W * noisy_ratio)

    key = (num, float(weight))
    if key not in _cache:
        _cache[key] = _build_program(num, weight)
    nc = _cache[key]

    ind_np, w16_np = _constants()
    in_maps = []
    for core in range(N_CORES):
        sl = slice(core * IMGS_PER_CORE, (core + 1) * IMGS_PER_CORE)
        in_maps.append({
            "gt": np.ascontiguousarray(gt_density[sl]),
            "dc": np.ascontiguousarray(dmap_conv[sl]),
            "dt": np.ascontiguousarray(dmap_tran[sl]),
            "ind": ind_np,
            "w16": w16_np,
        })

    res = run_bass_kernel_spmd(nc, in_maps, list(range(N_CORES)))
    total = np.float64(0.0)
    for core in range(N_CORES):
        total += res.results[core]["accp"].astype(np.float64).sum()
    return np.array(total, dtype=np.float32)


# revision 2
# speedup vs baseline: 1.0728x; 1.0728x over previous
"""CHSLoss (topk_masking) Trainium2 Bass kernel.

Data-parallel over batch: 8 cores x 4 images each. Per core:
  - 8x8 block-sum pooling of gt_density: f32 gt DMA'd in, cast to bf16 on the
    scalar engine, then PE matmuls with per-chunk block-indicator lhsT
    accumulate row-group sums into f32 PSUM; a strided DVE reduce finishes
    the column groups.
  - dg shuffled into a [16 partitions x 1024] per-image "row" layout so each
    loss row (image x {conv,tran}) owns a 16-partition group.
  - per-row top-k threshold via fixed-round bisection on squared errors E:
    count(E >= mid) is split between DVE (tensor_scalar is_ge + accum) and
    ACT (Sign(mid - E) + accum); partition-group sums via the DVE 32x32
    stream-transpose trick; fused scalar_tensor_tensor update chain.
  - masked MSE reduced to per-partition partials; host sums 8x128 partials.
"""

import numpy as np
import ml_dtypes

import concourse.bacc as bacc
import concourse.tile as tile
from concourse import mybir
from concourse.bass_utils import run_bass_kernel_spmd

F32 = mybir.dt.float32
BF16 = mybir.dt.bfloat16
ALU = mybir.AluOpType
AFT = mybir.ActivationFunctionType

N_CORES = 8
B, C, H, W = 32, 1, 128, 128
SIZE = 8
GH, GW = H * SIZE, W * SIZE  # 1024, 1024
IMGS_PER_CORE = B // N_CORES  # 4
MAX_NOISY_RATIO = 0.1
MAX_WEIGHT_RATIO = 1.0
R_BISECT = 22  # bisection rounds; rel err ~4e-6 (verified vs reference)

_cache: dict = {}


def _build_program(num: int, weight: float):
    nc = bacc.Bacc("TRN2", target_bir_lowering=False, debug=False,
                   num_devices=N_CORES)

    gt = nc.declare_dram_parameter("gt", [IMGS_PER_CORE, GH, GW], F32,
                                   isOutput=False)
    dcp = nc.declare_dram_parameter("dc", [IMGS_PER_CORE, H, W], F32,
                                    isOutput=False)
    dtp = nc.declare_dram_parameter("dt", [IMGS_PER_CORE, H, W], F32,
                                    isOutput=False)
    ind = nc.declare_dram_parameter("ind", [8 * 128, 128], BF16,
                                    isOutput=False)
    w16 = nc.declare_dram_parameter("w16", [128, 32], F32, isOutput=False)
    accp_out = nc.declare_dram_parameter("accp", [128, 1], F32, isOutput=True)

    with tile.TileContext(nc) as tc:
        with (
            tc.tile_pool(name="imgs", bufs=2) as imgpool,
            tc.tile_pool(name="imgsbf", bufs=2) as imgbfpool,
            tc.tile_pool(name="psum", bufs=2, space="PSUM") as psumpool,
            tc.tile_pool(name="consts", bufs=1) as constpool,
            tc.tile_pool(name="work", bufs=1) as work,
            tc.tile_pool(name="dg", bufs=2) as dgpool,
            tc.tile_pool(name="small", bufs=1) as small,
        ):
            indt = constpool.tile([128, 8, 128], BF16)
            nc.sync.dma_start(indt[:], ind.rearrange("(o k) m -> k o m", o=8))
            w16t = constpool.tile([128, 32], F32)
            nc.sync.dma_start(w16t[:], w16[:])

            G = work.tile([128, 8 * 128], F32)
            S = work.tile([128, 8 * 128], F32)
            Sp = work.tile([128, 8 * 128], F32)

            # dmap rows in [16, 1024] layout; tiny DMAs, issue them all early
            for i in range(IMGS_PER_CORE):
                dc_i = dcp[i].rearrange("(q r) w -> q r w", q=16)
                dt_i = dtp[i].rearrange("(q r) w -> q r w", q=16)
                for dst, src in (
                    (S[32 * i : 32 * i + 16, :], dc_i),
                    (S[32 * i + 16 : 32 * i + 32, :], dt_i),
                    (Sp[32 * i : 32 * i + 16, :], dt_i),
                    (Sp[32 * i + 16 : 32 * i + 32, :], dc_i),
                ):
                    nc.sync.dma_start(
                        dst.rearrange("q (r w) -> q r w", r=8), src)

            for i in range(IMGS_PER_CORE):
                # gt image: partition k holds rows {128*o + k}, free (o, w)
                img_t = imgpool.tile([128, 8, GW], F32)
                img_bf = imgbfpool.tile([128, 8, GW], BF16)
                gt_i = gt[i].rearrange("(o k) w -> k o w", o=8, k=128)
                for hf in range(2):
                    osl = slice(hf * 4, hf * 4 + 4)
                    nc.sync.dma_start(img_t[:, osl, :], gt_i[:, osl, :])
                    nc.scalar.activation(img_bf[:, osl, :], img_t[:, osl, :],
                                         AFT.Copy)

                # Stage A on PE (bf16 in, f32 PSUM accumulate): chunk o's
                # indicator places its 16 hh-groups at partitions [16o,16o+16)
                rs = psumpool.tile([128, GW], F32)
                for o in range(8):
                    for hf in range(2):
                        sl = slice(hf * 512, (hf + 1) * 512)
                        nc.tensor.matmul(
                            rs[:, sl],
                            indt[:, o, :],
                            img_bf[:, o, sl],
                            start=(o == 0),
                            stop=(o == 7),
                        )

                # Stage B: sum each 8-wide column group -> dg [128(hh),128(ww)]
                dg = dgpool.tile([128, 128], F32)
                nc.vector.tensor_reduce(
                    dg[:],
                    rs[:].rearrange("p (w j) -> p w j", j=8),
                    axis=mybir.AxisListType.X,
                    op=ALU.add,
                )

                # Shuffle dg -> G row block: G[32i+q, r*128+w] = dg[8q+r, w]
                gslot = G[32 * i : 32 * i + 16, :]
                for r in range(8):
                    nc.sync.dma_start(
                        gslot[:, r * 128 : (r + 1) * 128],
                        dg[r : r + 121 : 8, :],
                    )
                nc.sync.dma_start(G[32 * i + 16 : 32 * i + 32, :], gslot[:])

            # A = S - G ; Bw = weight * (Sp - G) ; E = A*A
            A = work.tile([128, 1024], F32)
            Bw = work.tile([128, 1024], F32)
            E = work.tile([128, 1024], F32)
            nc.vector.tensor_tensor(out=A[:], in0=S[:], in1=G[:],
                                    op=ALU.subtract)
            nc.vector.tensor_tensor(out=Bw[:], in0=Sp[:], in1=G[:],
                                    op=ALU.subtract)
            if weight != 1.0:
                nc.vector.tensor_scalar(out=Bw[:], in0=Bw[:],
                                        scalar1=float(weight), scalar2=None,
                                        op0=ALU.mult)
            nc.vector.tensor_tensor(out=E[:], in0=A[:], in1=A[:], op=ALU.mult)

            cjunk = work.tile([128, 1024], F32)
            sjunk = work.tile([128, 512], F32)
            thr = small.tile([128, 1], F32)
            if num >= 1:
                mid = small.tile([128, 1], F32)
                cnt_d = small.tile([128, 1], F32)
                acc_a = small.tile([128, 1], F32)
                s_t = small.tile([128, 1], F32)
                tj = small.tile([128, 32], F32)
                gj = small.tile([128, 32], F32)
                gcnt = small.tile([128, 1], F32)
                delta = small.tile([128, 1], F32)
                nc.vector.memset(mid[:], 2048.0)
                for r in range(R_BISECT):
                    # count(E >= mid): DVE on cols [0:512], ACT on [512:1024]
                    nc.vector.tensor_scalar(
                        out=cjunk[:, 0:512], in0=E[:, 0:512], scalar1=mid[:],
                        scalar2=0.0, op0=ALU.is_ge, op1=ALU.add,
                        accum_out=cnt_d[:],
                    )
                    # Sign(mid - E) summed: count_ge = (512 - acc_a) / 2
                    nc.scalar.activation(
                        sjunk[:], E[:, 512:1024], AFT.Sign,
                        bias=mid[:], scale=-1.0, accum_out=acc_a[:],
                    )
                    # s = cnt_d - 0.5*acc_a  (= per-partition count - 256)
                    nc.vector.scalar_tensor_tensor(
                        out=s_t[:], in0=acc_a[:], scalar=-0.5, in1=cnt_d[:],
                        op0=ALU.mult, op1=ALU.add,
                    )
                    # group stat: sum s over 16-partition blocks, bcast back
                    nc.vector.transpose(tj[:], s_t[:].to_broadcast([128, 32]))
                    nc.vector.scalar_tensor_tensor(
                        out=gj[:], in0=tj[:], scalar=0.0, in1=w16t[:],
                        op0=ALU.add, op1=ALU.mult, accum_out=gcnt[:],
                    )
                    step = float(1024.0 * 2.0 ** (-r))
                    # sel = (gcnt >= num - 4096); mid += sel*2*step - step
                    nc.vector.tensor_scalar(
                        out=delta[:], in0=gcnt[:], scalar1=float(num - 4096),
                        scalar2=2.0 * step, op0=ALU.is_ge, op1=ALU.mult,
                    )
                    nc.vector.scalar_tensor_tensor(
                        out=mid[:], in0=mid[:], scalar=-step, in1=delta[:],
                        op0=ALU.add, op1=ALU.add,
                    )
                nc.vector.tensor_scalar(
                    out=thr[:], in0=mid[:],
                    scalar1=float(4096.0 * 2.0 ** (-R_BISECT)), scalar2=None,
                    op0=ALU.subtract,
                )
            else:
                nc.vector.memset(thr[:], 3.0e38)

            # t1 = (E >= thr) * Bw ; d = A - t1 ; accp = sum(d*d) per part
            t1 = work.tile([128, 1024], F32)
            nc.vector.scalar_tensor_tensor(
                out=t1[:], in0=E[:], scalar=thr[:], in1=Bw[:],
                op0=ALU.is_ge, op1=ALU.mult,
            )
            d = cjunk  # reuse
            nc.vector.tensor_tensor(out=d[:], in0=A[:], in1=t1[:],
                                    op=ALU.subtract)
            dsq = t1  # reuse
            accp = small.tile([128, 1], F32)
            nc.vector.scalar_tensor_tensor(
                out=dsq[:], in0=d[:], scalar=0.0, in1=d[:],
                op0=ALU.add, op1=ALU.mult, accum_out=accp[:],
            )
            nc.sync.dma_start(accp_out[:], accp[:])

    nc.compile()
    return nc


def _constants():
    ind_np = np.zeros((8, 128, 128), dtype=np.float32)
    for o in range(8):
        for k in range(128):
            ind_np[o, k, 16 * o + k // 8] = 1.0
    w16_np = np.zeros((128, 32), dtype=np.float32)
    for p in range(128):
        w16_np[p, 16 * ((p // 16) % 2) : 16 * ((p // 16) % 2) + 16] = 1.0
    return (
        ind_np.reshape(8 * 128, 128).astype(ml_dtypes.bfloat16),
        w16_np,
    )


def kernel(dmap_conv, dmap_tran, gt_density, process):
    dmap_conv = np.asarray(dmap_conv, dtype=np.float32).reshape(B, H, W)
    dmap_tran = np.asarray(dmap_tran, dtype=np.float32).reshape(B, H, W)
    gt_density = np.asarray(gt_density, dtype=np.float32).reshape(B, GH, GW)
    p = float(np.asarray(process))

    weight = MAX_WEIGHT_RATIO * p
    noisy_ratio = MAX_NOISY_RATIO * p
    num = int(H * W * noisy_ratio)

    key = (num, float(weight))
    if key not in _cache:
        _cache[key] = _build_program(num, weight)
    nc = _cache[key]

    ind_np, w16_np = _constants()
    in_maps = []
    for core in range(N_CORES):
        sl = slice(core * IMGS_PER_CORE, (core + 1) * IMGS_PER_CORE)
        in_maps.append({
            "gt": np.ascontiguousarray(gt_density[sl]),
            "dc": np.ascontiguousarray(dmap_conv[sl]),
            "dt": np.ascontiguousarray(dmap_tran[sl]),
            "ind": ind_np,
            "w16": w16_np,
        })

    res = run_bass_kernel_spmd(nc, in_maps, list(range(N_CORES)))
    total = np.float64(0.0)
    for core in range(N_CORES):
        total += res.results[core]["accp"].astype(np.float64).sum()
    return np.array(total, dtype=np.float32)


# revision 4
# speedup vs baseline: 1.1985x; 1.1171x over previous
"""CHSLoss (topk_masking) Trainium2 Bass kernel.

Data-parallel over batch: 8 cores x 4 images each. Per core:
  - 8x8 block-sum pooling of gt_density: f32 gt DMA'd in, cast to bf16 on the
    scalar engine, then PE matmuls with per-chunk block-indicator lhsT
    accumulate row-group sums into f32 PSUM; a strided DVE reduce finishes
    the column groups.
  - dg shuffled into a [16 partitions x 1024] per-image "row" layout so each
    loss row (image x {conv,tran}) owns a 16-partition group.
  - per-row top-k threshold via fixed-round bisection on squared errors E:
    count(E >= mid) is split between DVE (tensor_scalar is_ge + accum) and
    ACT (Sign(mid - E) + accum); partition-group sums via the DVE 32x32
    stream-transpose trick; fused scalar_tensor_tensor update chain.
  - masked MSE reduced to per-partition partials; host sums 8x128 partials.
"""

import numpy as np
import ml_dtypes

import concourse.bacc as bacc
import concourse.tile as tile
from concourse import mybir
from concourse.bass_utils import run_bass_kernel_spmd

F32 = mybir.dt.float32
BF16 = mybir.dt.bfloat16
ALU = mybir.AluOpType
AFT = mybir.ActivationFunctionType

N_CORES = 8
B, C, H, W = 32, 1, 128, 128
SIZE = 8
GH, GW = H * SIZE, W * SIZE  # 1024, 1024
IMGS_PER_CORE = B // N_CORES  # 4
MAX_NOISY_RATIO = 0.1
MAX_WEIGHT_RATIO = 1.0
R_BISECT = 18  # bisection rounds; rel err ~3e-5 (verified vs reference)

_cache: dict = {}


def _build_program(num: int, weight: float):
    nc = bacc.Bacc("TRN2", target_bir_lowering=False, debug=False,
                   num_devices=N_CORES)

    gt = nc.declare_dram_parameter("gt", [IMGS_PER_CORE, GH, GW], F32,
                                   isOutput=False)
    dcp = nc.declare_dram_parameter("dc", [IMGS_PER_CORE, H, W], F32,
                                    isOutput=False)
    dtp = nc.declare_dram_parameter("dt", [IMGS_PER_CORE, H, W], F32,
                                    isOutput=False)
    ind = nc.declare_dram_parameter("ind", [8 * 128, 128], BF16,
                                    isOutput=False)
    w16 = nc.declare_dram_parameter("w16", [128, 32], F32, isOutput=False)
    accp_out = nc.declare_dram_parameter("accp", [128, 1], F32, isOutput=True)

    with tile.TileContext(nc) as tc:
        with (
            tc.tile_pool(name="imgs", bufs=2) as imgpool,
            tc.tile_pool(name="imgsbf", bufs=2) as imgbfpool,
            tc.tile_pool(name="psum", bufs=2, space="PSUM") as psumpool,
            tc.tile_pool(name="consts", bufs=1) as constpool,
            tc.tile_pool(name="work", bufs=1) as work,
            tc.tile_pool(name="dg", bufs=2) as dgpool,
            tc.tile_pool(name="small", bufs=1) as small,
        ):
            indt = constpool.tile([128, 8, 128], BF16)
            nc.sync.dma_start(indt[:], ind.rearrange("(o k) m -> k o m", o=8))
            w16t = constpool.tile([128, 32], F32)
            nc.sync.dma_start(w16t[:], w16[:])

            G = work.tile([128, 8 * 128], F32)
            S = work.tile([128, 8 * 128], F32)
            Sp = work.tile([128, 8 * 128], F32)

            # dmap rows in [16, 1024] layout; tiny DMAs, issue them all early
            for i in range(IMGS_PER_CORE):
                dc_i = dcp[i].rearrange("(q r) w -> q r w", q=16)
                dt_i = dtp[i].rearrange("(q r) w -> q r w", q=16)
                for dst, src in (
                    (S[32 * i : 32 * i + 16, :], dc_i),
                    (S[32 * i + 16 : 32 * i + 32, :], dt_i),
                    (Sp[32 * i : 32 * i + 16, :], dt_i),
                    (Sp[32 * i + 16 : 32 * i + 32, :], dc_i),
                ):
                    nc.gpsimd.dma_start(
                        dst.rearrange("q (r w) -> q r w", r=8), src)

            for i in range(IMGS_PER_CORE):
                # gt image: partition k holds rows {128*o + k}, free (o, w)
                img_t = imgpool.tile([128, 8, GW], F32)
                img_bf = imgbfpool.tile([128, 8, GW], BF16)
                gt_i = gt[i].rearrange("(o k) w -> k o w", o=8, k=128)
                for q in range(4):
                    osl = slice(q * 2, q * 2 + 2)
                    eng = nc.sync if q % 2 == 0 else nc.scalar
                    eng.dma_start(img_t[:, osl, :], gt_i[:, osl, :])
                    nc.scalar.activation(img_bf[:, osl, :], img_t[:, osl, :],
                                         AFT.Copy)

                # Stage A on PE (bf16 in, f32 PSUM accumulate): chunk o's
                # indicator places its 16 hh-groups at partitions [16o,16o+16)
                rs = psumpool.tile([128, GW], F32)
                for o in range(8):
                    for hf in range(2):
                        sl = slice(hf * 512, (hf + 1) * 512)
                        nc.tensor.matmul(
                            rs[:, sl],
                            indt[:, o, :],
                            img_bf[:, o, sl],
                            start=(o == 0),
                            stop=(o == 7),
                        )

                # Stage B: sum each 8-wide column group -> dg [128(hh),128(ww)]
                dg = dgpool.tile([128, 128], F32)
                nc.vector.tensor_reduce(
                    dg[:],
                    rs[:].rearrange("p (w j) -> p w j", j=8),
                    axis=mybir.AxisListType.X,
                    op=ALU.add,
                )

                # Shuffle dg -> G row block: G[32i+q, r*128+w] = dg[8q+r, w]
                gslot = G[32 * i : 32 * i + 16, :]
                for r in range(8):
                    nc.gpsimd.dma_start(
                        gslot[:, r * 128 : (r + 1) * 128],
                        dg[r : r + 121 : 8, :],
                    )
                nc.gpsimd.dma_start(G[32 * i + 16 : 32 * i + 32, :], gslot[:])

            # A = S - G ; Bw = weight * (Sp - G) ; E = A*A
            A = work.tile([128, 1024], F32)
            Bw = work.tile([128, 1024], F32)
            E = work.tile([128, 1024], F32)
            nc.vector.tensor_tensor(out=A[:], in0=S[:], in1=G[:],
                                    op=ALU.subtract)
            nc.vector.tensor_tensor(out=Bw[:], in0=Sp[:], in1=G[:],
                                    op=ALU.subtract)
            if weight != 1.0:
                nc.vector.tensor_scalar(out=Bw[:], in0=Bw[:],
                                        scalar1=float(weight), scalar2=None,
                                        op0=ALU.mult)
            nc.vector.tensor_tensor(out=E[:], in0=A[:], in1=A[:], op=ALU.mult)

            cjunk = work.tile([128, 1024], F32)
            sjunk = work.tile([128, 512], F32)
            thr = small.tile([128, 1], F32)
            if num >= 1:
                mid = small.tile([128, 1], F32)
                cnt_d = small.tile([128, 1], F32)
                acc_a = small.tile([128, 1], F32)
                s_t = small.tile([128, 1], F32)
                tj = small.tile([128, 32], F32)
                gj = small.tile([128, 32], F32)
                gcnt = small.tile([128, 1], F32)
                delta = small.tile([128, 1], F32)
                nc.vector.memset(mid[:], 2048.0)
                for r in range(R_BISECT):
                    # count(E >= mid): DVE on cols [0:512], ACT on [512:1024]
                    nc.vector.tensor_scalar(
                        out=cjunk[:, 0:576], in0=E[:, 0:576], scalar1=mid[:],
                        scalar2=0.0, op0=ALU.is_ge, op1=ALU.add,
                        accum_out=cnt_d[:],
                    )
                    # Sign(mid - E) summed: count_ge = (512 - acc_a) / 2
                    nc.scalar.activation(
                        sjunk[:, 0:448], E[:, 576:1024], AFT.Sign,
                        bias=mid[:], scale=-1.0, accum_out=acc_a[:],
                    )
                    # s = cnt_d - 0.5*acc_a  (= per-partition count - 256)
                    nc.vector.scalar_tensor_tensor(
                        out=s_t[:], in0=acc_a[:], scalar=-0.5, in1=cnt_d[:],
                        op0=ALU.mult, op1=ALU.add,
                    )
                    # group stat: sum s over 16-partition blocks, bcast back
                    nc.vector.transpose(tj[:], s_t[:].to_broadcast([128, 32]))
                    nc.vector.scalar_tensor_tensor(
                        out=gj[:], in0=tj[:], scalar=0.0, in1=w16t[:],
                        op0=ALU.add, op1=ALU.mult, accum_out=gcnt[:],
                    )
                    step = float(1024.0 * 2.0 ** (-r))
                    # sel = (gcnt >= num - 4096); mid += sel*2*step - step
                    nc.vector.tensor_scalar(
                        out=delta[:], in0=gcnt[:], scalar1=float(num - 16 * 224),
                        scalar2=2.0 * step, op0=ALU.is_ge, op1=ALU.mult,
                    )
                    nc.vector.scalar_tensor_tensor(
                        out=mid[:], in0=mid[:], scalar=-step, in1=delta[:],
                        op0=ALU.add, op1=ALU.add,
                    )
                nc.vector.tensor_scalar(
                    out=thr[:], in0=mid[:],
                    scalar1=float(4096.0 * 2.0 ** (-R_BISECT)), scalar2=None,
                    op0=ALU.subtract,
                )
            else:
                nc.vector.memset(thr[:], 3.0e38)

            # t1 = (E >= thr) * Bw ; d = A - t1 ; accp = sum(d*d) per part
            t1 = work.tile([128, 1024], F32)
            nc.vector.scalar_tensor_tensor(
                out=t1[:], in0=E[:], scalar=thr[:], in1=Bw[:],
                op0=ALU.is_ge, op1=ALU.mult,
            )
            d = cjunk  # reuse
            nc.vector.tensor_tensor(out=d[:], in0=A[:], in1=t1[:],
                                    op=ALU.subtract)
            dsq = t1  # reuse
            accp = small.tile([128, 1], F32)
            nc.vector.scalar_tensor_tensor(
                out=dsq[:], in0=d[:], scalar=0.0, in1=d[:],
                op0=ALU.add, op1=ALU.mult, accum_out=accp[:],
            )
            nc.sync.dma_start(accp_out[:], accp[:])

    nc.compile()
    return nc


def _constants():
    ind_np = np.zeros((8, 128, 128), dtype=np.float32)
    for o in range(8):
        for k in range(128):
            ind_np[o, k, 16 * o + k // 8] = 1.0
    w16_np = np.zeros((128, 32), dtype=np.float32)
    for p in range(128):
        w16_np[p, 16 * ((p // 16) % 2) : 16 * ((p // 16) % 2) + 16] = 1.0
    return (
        ind_np.reshape(8 * 128, 128).astype(ml_dtypes.bfloat16),
        w16_np,
    )


def kernel(dmap_conv, dmap_tran, gt_density, process):
    dmap_conv = np.asarray(dmap_conv, dtype=np.float32).reshape(B, H, W)
    dmap_tran = np.asarray(dmap_tran, dtype=np.float32).reshape(B, H, W)
    gt_density = np.asarray(gt_density, dtype=np.float32).reshape(B, GH, GW)
    p = float(np.asarray(process))

    weight = MAX_WEIGHT_RATIO * p
    noisy_ratio = MAX_NOISY_RATIO * p
    num = int(H * W * noisy_ratio)

    key = (num, float(weight))
    if key not in _cache:
        _cache[key] = _build_program(num, weight)
    nc = _cache[key]

    ind_np, w16_np = _constants()
    in_maps = []
    for core in range(N_CORES):
        sl = slice(core * IMGS_PER_CORE, (core + 1) * IMGS_PER_CORE)
        in_maps.append({
            "gt": np.ascontiguousarray(gt_density[sl]),
            "dc": np.ascontiguousarray(dmap_conv[sl]),
            "dt": np.ascontiguousarray(dmap_tran[sl]),
            "ind": ind_np,
            "w16": w16_np,
        })

    res = run_bass_kernel_spmd(nc, in_maps, list(range(N_CORES)))
    total = np.float64(0.0)
    for core in range(N_CORES):
        total += res.results[core]["accp"].astype(np.float64).sum()
    return np.array(total, dtype=np.float32)


# revision 6
# speedup vs baseline: 1.3131x; 1.0956x over previous
"""CHSLoss (topk_masking) Trainium2 Bass kernel.

Data-parallel over batch: 8 cores x 4 images each. Per core:
  - 8x8 block-sum pooling of gt_density: f32 gt DMA'd in, cast to bf16 on the
    scalar engine, then PE matmuls with per-chunk block-indicator lhsT
    accumulate row-group sums into f32 PSUM; a strided DVE reduce finishes
    the column groups.
  - dg shuffled into a [16 partitions x 1024] per-image "row" layout so each
    loss row (image x {conv,tran}) owns a 16-partition group.
  - per-row top-k threshold via fixed-round bisection on squared errors E:
    count(E >= mid) is split between DVE (tensor_scalar is_ge + accum) and
    ACT (Sign(mid - E) + accum); partition-group sums via the DVE 32x32
    stream-transpose trick; fused scalar_tensor_tensor update chain.
  - masked MSE reduced to per-partition partials; host sums 8x128 partials.
"""

import numpy as np
import ml_dtypes

import concourse.bacc as bacc
import concourse.tile as tile
from concourse import mybir
from concourse.bass_utils import run_bass_kernel_spmd

F32 = mybir.dt.float32
F32R = mybir.dt.float32r
BF16 = mybir.dt.bfloat16
ALU = mybir.AluOpType
AFT = mybir.ActivationFunctionType

N_CORES = 8
B, C, H, W = 32, 1, 128, 128
SIZE = 8
GH, GW = H * SIZE, W * SIZE  # 1024, 1024
IMGS_PER_CORE = B // N_CORES  # 4
MAX_NOISY_RATIO = 0.1
MAX_WEIGHT_RATIO = 1.0
R_BISECT = 18  # bisection rounds; rel err ~3e-5 (verified vs reference)

_cache: dict = {}


def _build_program(num: int, weight: float):
    nc = bacc.Bacc("TRN2", target_bir_lowering=False, debug=False,
                   num_devices=N_CORES)

    gt = nc.declare_dram_parameter("gt", [IMGS_PER_CORE, GH, GW], F32R,
                                   isOutput=False)
    dcp = nc.declare_dram_parameter("dc", [IMGS_PER_CORE, H, W], F32,
                                    isOutput=False)
    dtp = nc.declare_dram_parameter("dt", [IMGS_PER_CORE, H, W], F32,
                                    isOutput=False)
    ind = nc.declare_dram_parameter("ind", [8 * 128, 128], F32R,
                                    isOutput=False)
    w16 = nc.declare_dram_parameter("w16", [128, 32], F32, isOutput=False)
    accp_out = nc.declare_dram_parameter("accp", [128, 1], F32, isOutput=True)

    with tile.TileContext(nc) as tc:
        with (
            tc.tile_pool(name="imgs", bufs=2) as imgpool,
            tc.tile_pool(name="psum", bufs=2, space="PSUM") as psumpool,
            tc.tile_pool(name="consts", bufs=1) as constpool,
            tc.tile_pool(name="work", bufs=1) as work,
            tc.tile_pool(name="dg", bufs=2) as dgpool,
            tc.tile_pool(name="small", bufs=1) as small,
        ):
            indt = constpool.tile([128, 8, 128], F32R)
            nc.sync.dma_start(indt[:], ind.rearrange("(o k) m -> k o m", o=8))
            w16t = constpool.tile([128, 32], F32)
            nc.sync.dma_start(w16t[:], w16[:])

            G = work.tile([128, 8 * 128], F32)
            S = work.tile([128, 8 * 128], F32)
            Sp = work.tile([128, 8 * 128], F32)

            # dmap rows in [16, 1024] layout; tiny DMAs, issue them all early
            for i in range(IMGS_PER_CORE):
                dc_i = dcp[i].rearrange("(q r) w -> q r w", q=16)
                dt_i = dtp[i].rearrange("(q r) w -> q r w", q=16)
                for dst, src in (
                    (S[32 * i : 32 * i + 16, :], dc_i),
                    (S[32 * i + 16 : 32 * i + 32, :], dt_i),
                    (Sp[32 * i : 32 * i + 16, :], dt_i),
                    (Sp[32 * i + 16 : 32 * i + 32, :], dc_i),
                ):
                    nc.gpsimd.dma_start(
                        dst.rearrange("q (r w) -> q r w", r=8), src)

            for i in range(IMGS_PER_CORE):
                # gt image: partition k holds rows {128*o + k}, free (o, w)
                img_t = imgpool.tile([128, 8, GW], F32R)
                gt_i = gt[i].rearrange("(o k) w -> k o w", o=8, k=128)
                for q in range(4):
                    osl = slice(q * 2, q * 2 + 2)
                    eng = nc.sync if q % 2 == 0 else nc.scalar
                    eng.dma_start(img_t[:, osl, :], gt_i[:, osl, :])

                # Stage A on PE (bf16 in, f32 PSUM accumulate): chunk o's
                # indicator places its 16 hh-groups at partitions [16o,16o+16)
                rs = psumpool.tile([128, GW], F32)
                for o in range(8):
                    for hf in range(2):
                        sl = slice(hf * 512, (hf + 1) * 512)
                        nc.tensor.matmul(
                            rs[:, sl],
                            indt[:, o, :],
                            img_t[:, o, sl],
                            start=(o == 0),
                            stop=(o == 7),
                        )

                # Stage B: sum each 8-wide column group -> dg [128(hh),128(ww)]
                dg = dgpool.tile([128, 128], F32)
                nc.vector.tensor_reduce(
                    dg[:],
                    rs[:].rearrange("p (w j) -> p w j", j=8),
                    axis=mybir.AxisListType.X,
                    op=ALU.add,
                )

                # Shuffle dg -> G row block: G[32i+q, r*128+w] = dg[8q+r, w]
                gslot = G[32 * i : 32 * i + 16, :]
                for r in range(8):
                    nc.gpsimd.dma_start(
                        gslot[:, r * 128 : (r + 1) * 128],
                        dg[r : r + 121 : 8, :],
                    )
                nc.gpsimd.dma_start(G[32 * i + 16 : 32 * i + 32, :], gslot[:])

            # A = S - G ; Bw = weight * (Sp - G) ; E = A*A
            A = work.tile([128, 1024], F32)
            Bw = work.tile([128, 1024], F32)
            E = work.tile([128, 1024], F32)
            nc.vector.tensor_tensor(out=A[:], in0=S[:], in1=G[:],
                                    op=ALU.subtract)
            nc.vector.tensor_tensor(out=Bw[:], in0=Sp[:], in1=G[:],
                                    op=ALU.subtract)
            if weight != 1.0:
                nc.vector.tensor_scalar(out=Bw[:], in0=Bw[:],
                                        scalar1=float(weight), scalar2=None,
                                        op0=ALU.mult)
            nc.vector.tensor_tensor(out=E[:], in0=A[:], in1=A[:], op=ALU.mult)

            cjunk = work.tile([128, 1024], F32)
            sjunk = work.tile([128, 512], F32)
            thr = small.tile([128, 1], F32)
            if num >= 1:
                mid = small.tile([128, 1], F32)
                cnt_d = small.tile([128, 1], F32)
                acc_a = small.tile([128, 1], F32)
                s_t = small.tile([128, 1], F32)
                tj = small.tile([128, 32], F32)
                gj = small.tile([128, 32], F32)
                gcnt = small.tile([128, 1], F32)
                delta = small.tile([128, 1], F32)
                nc.vector.memset(mid[:], 2048.0)
                for r in range(R_BISECT):
                    # count(E >= mid): DVE on cols [0:512], ACT on [512:1024]
                    nc.vector.tensor_scalar(
                        out=cjunk[:, 0:576], in0=E[:, 0:576], scalar1=mid[:],
                        scalar2=0.0, op0=ALU.is_ge, op1=ALU.add,
                        accum_out=cnt_d[:],
                    )
                    # Sign(mid - E) summed: count_ge = (512 - acc_a) / 2
                    nc.scalar.activation(
                        sjunk[:, 0:448], E[:, 576:1024], AFT.Sign,
                        bias=mid[:], scale=-1.0, accum_out=acc_a[:],
                    )
                    # s = cnt_d - 0.5*acc_a  (= per-partition count - 256)
                    nc.vector.scalar_tensor_tensor(
                        out=s_t[:], in0=acc_a[:], scalar=-0.5, in1=cnt_d[:],
                        op0=ALU.mult, op1=ALU.add,
                    )
                    # group stat: sum s over 16-partition blocks, bcast back
                    nc.vector.transpose(tj[:], s_t[:].to_broadcast([128, 32]))
                    nc.vector.scalar_tensor_tensor(
                        out=gj[:], in0=tj[:], scalar=0.0, in1=w16t[:],
                        op0=ALU.add, op1=ALU.mult, accum_out=gcnt[:],
                    )
                    step = float(1024.0 * 2.0 ** (-r))
                    # sel = (gcnt >= num - 4096); mid += sel*2*step - step
                    nc.vector.tensor_scalar(
                        out=delta[:], in0=gcnt[:], scalar1=float(num - 16 * 224),
                        scalar2=2.0 * step, op0=ALU.is_ge, op1=ALU.mult,
                    )
                    nc.vector.scalar_tensor_tensor(
                        out=mid[:], in0=mid[:], scalar=-step, in1=delta[:],
                        op0=ALU.add, op1=ALU.add,
                    )
                nc.vector.tensor_scalar(
                    out=thr[:], in0=mid[:],
                    scalar1=float(4096.0 * 2.0 ** (-R_BISECT)), scalar2=None,
                    op0=ALU.subtract,
                )
            else:
                nc.vector.memset(thr[:], 3.0e38)

            # t1 = (E >= thr) * Bw ; d = A - t1 ; accp = sum(d*d) per part
            t1 = work.tile([128, 1024], F32)
            nc.vector.scalar_tensor_tensor(
                out=t1[:], in0=E[:], scalar=thr[:], in1=Bw[:],
                op0=ALU.is_ge, op1=ALU.mult,
            )
            d = cjunk  # reuse
            nc.vector.tensor_tensor(out=d[:], in0=A[:], in1=t1[:],
                                    op=ALU.subtract)
            dsq = t1  # reuse
            accp = small.tile([128, 1], F32)
            nc.vector.scalar_tensor_tensor(
                out=dsq[:], in0=d[:], scalar=0.0, in1=d[:],
                op0=ALU.add, op1=ALU.mult, accum_out=accp[:],
            )
            nc.sync.dma_start(accp_out[:], accp[:])

    nc.compile()
    return nc


def _constants():
    ind_np = np.zeros((8, 128, 128), dtype=np.float32)
    for o in range(8):
        for k in range(128):
            ind_np[o, k, 16 * o + k // 8] = 1.0
    w16_np = np.zeros((128, 32), dtype=np.float32)
    for p in range(128):
        w16_np[p, 16 * ((p // 16) % 2) : 16 * ((p // 16) % 2) + 16] = 1.0
    return (
        ind_np.reshape(8 * 128, 128),
        w16_np,
    )


def kernel(dmap_conv, dmap_tran, gt_density, process):
    dmap_conv = np.asarray(dmap_conv, dtype=np.float32).reshape(B, H, W)
    dmap_tran = np.asarray(dmap_tran, dtype=np.float32).reshape(B, H, W)
    gt_density = np.asarray(gt_density, dtype=np.float32).reshape(B, GH, GW)
    p = float(np.asarray(process))

    weight = MAX_WEIGHT_RATIO * p
    noisy_ratio = MAX_NOISY_RATIO * p
    num = int(H * W * noisy_ratio)

    key = (num, float(weight))
    if key not in _cache:
        _cache[key] = _build_program(num, weight)
    nc = _cache[key]

    ind_np, w16_np = _constants()
    in_maps = []
    for core in range(N_CORES):
        sl = slice(core * IMGS_PER_CORE, (core + 1) * IMGS_PER_CORE)
        in_maps.append({
            "gt": np.ascontiguousarray(gt_density[sl]),
            "dc": np.ascontiguousarray(dmap_conv[sl]),
            "dt": np.ascontiguousarray(dmap_tran[sl]),
            "ind": ind_np,
            "w16": w16_np,
        })

    res = run_bass_kernel_spmd(nc, in_maps, list(range(N_CORES)))
    total = np.float64(0.0)
    for core in range(N_CORES):
        total += res.results[core]["accp"].astype(np.float64).sum()
    return np.array(total, dtype=np.float32)
